# revision 12
# baseline (speedup 1.0000x reference)
# Trainium2 Bass kernel for nn_FCM_series_1 (gnn_message_passing).
#
# Math (derived from the reference):
#   aggregate(X, WW)[l,b,j] = tanh(-sum_i X[l,b,i] * WW[i,j])
#   T_A  = aggregate(A, WW)                     (12 lags x B rows)
#   U[t] = aggregate(train_init[:,:,t,1], WW)   (13 unique rows per batch;
#          A_N_OLD[la] = U[la], A_0_NEW[la] = U[la+1])
#   out[b,la,j] = P[la,j]*T_A[la,b,j] + Q[la,j]*U[la+1,b,j] + R[la,j]*U[la,b,j]
# with host-computable coefficients
#   P[la,j] = 2 * lambd[la, j%200] / belta[la] * 3**fract[la]
#   Q[la,j] = 3 * lambd[la, j%200] * l[la, j%200] / belta[la]
#   R[la,j] = Q[la,j] * Gamma(a+1)/(6*Gamma(a-2))
#   belta[la] = sum_{k=0..3} Gamma(a+1)/(Gamma(k+1)*Gamma(a-k+1))
#
# Sharding over 8 cores: batch split x2 (16 each), output node dim j split x4
# (300 each). Per core one matmul chain: lhsT=W-chunk tiles, rhs=X^T tiles,
# PSUM-accumulated over 10 k-tiles of 120, in float32r (single-pass fp32 PE
# mode, 4x faster than fp32 LOW_HIGH). W is negated on the host so psum
# already holds -X@W; tanh on ScalarE; coefficient combine on VectorE with
# 0-stride broadcast APs; per-core [300,192] result re-assembled on the host.
#
# HBM layouts are host-repacked to partition-major so every DMA descriptor is
# one large contiguous run per partition; input DMAs are split between the two
# HWDGE queues (sync for W, scalar for X) to double aggregate DMA throughput.

import math

import numpy as np

LAG = 13
B = 32
N = 1200
H = 1.0 / 3.0

PB = 2          # batch shards
PJ = 4          # j shards
BL = B // PB    # 16 batches per core
JL = N // PJ    # 300 output nodes per core
NL = LAG - 1    # 12
CA = NL * BL    # 192 cols: T_A block, col = la*BL + b
CU = LAG * BL   # 208 cols: U block,  col = CA + t*BL + b
C = CA + CU     # 400 matmul moving cols
KT = 120        # contraction tile
NK = N // KT    # 10
JS = 100        # j subtile (psum partition dim)
NJ = JL // JS   # 3
NCH = 2         # input DMA chunks per tensor (5 k-tiles each)

_cached = None


def _gamma(x):
    return math.gamma(x)


def _build_nc():
    import concourse.bacc as bacc
    import concourse.mybir as mybir
    from concourse.tile import TileContext

    f32 = mybir.dt.float32
    f32r = mybir.dt.float32r
    nc = bacc.Bacc(None, target_bir_lowering=False)

    # partition-major repacked inputs (see kernel() for layouts)
    xt = nc.dram_tensor("xt", [KT, NK * C], f32r, kind="ExternalInput")
    wc = nc.dram_tensor("wc", [KT, NK * JL], f32r, kind="ExternalInput")
    coef = nc.dram_tensor("coef", [JS, 3 * NJ * NL], f32, kind="ExternalInput")
    out = nc.dram_tensor("out", [JL, CA], f32, kind="ExternalOutput")

    with TileContext(nc) as tc:
        with (
            tc.tile_pool(name="sb", bufs=1) as pool,
            tc.tile_pool(name="ps", bufs=1, space="PSUM") as pspool,
        ):
            KPC = NK // NCH          # k-tiles per DMA chunk (2)
            wch = KPC * JL
            xch = KPC * C
            wt, xtt = [], []
            for i in range(NCH):
                w_tile = pool.tile([KT, wch], f32r, tag="w", bufs=NCH,
                                   name=f"w{i}")
                nc.sync.dma_start(
                    out=w_tile[:], in_=wc[:, i * wch:(i + 1) * wch])
                wt.append(w_tile)
                x_tile = pool.tile([KT, xch], f32r, tag="x", bufs=NCH,
                                   name=f"x{i}")
                nc.scalar.dma_start(
                    out=x_tile[:], in_=xt[:, i * xch:(i + 1) * xch])
                xtt.append(x_tile)
            coef_all = pool.tile([JS, 3 * NJ * NL], f32, tag="coef")
            nc.gpsimd.dma_start(out=coef_all[:], in_=coef[:, :])

            ps = [pspool.tile([JS, C], f32, tag=f"ps{jt}", name=f"ps{jt}")
                  for jt in range(NJ)]
            for k in range(NK):
                ci, kk = divmod(k, KPC)
                for jt in range(NJ):
                    nc.tensor.matmul(
                        ps[jt][:],
                        wt[ci][:, kk * JL + jt * JS:kk * JL + (jt + 1) * JS],
                        xtt[ci][:, kk * C:(kk + 1) * C],
                        start=(k == 0),
                        stop=(k == NK - 1),
                    )

            t_all = pool.tile([JS, NJ * C], f32, tag="t")
            for jt in range(NJ):
                # W was negated on the host, so psum = -(X @ W) already.
                nc.scalar.activation(
                    out=t_all[:, jt * C:(jt + 1) * C], in_=ps[jt][:],
                    func=mybir.ActivationFunctionType.Tanh,
                )

            res = pool.tile([JS, NJ * CA], f32, tag="res")
            tmp = pool.tile([JS, NJ * CA], f32, tag="tmp")
            t3 = t_all[:, :].rearrange("p (j c) -> p j c", j=NJ)
            tA = t3[:, :, 0:CA].rearrange("p j (l b) -> p j l b", b=BL)
            tU1 = t3[:, :, CA + BL:CA + CU].rearrange(
                "p j (l b) -> p j l b", b=BL)
            tU0 = t3[:, :, CA:CA + CA].rearrange("p j (l b) -> p j l b", b=BL)
            resv = res[:, :].rearrange("p (j l b) -> p j l b", j=NJ, b=BL)
            tmpv = tmp[:, :].rearrange("p (j l b) -> p j l b", j=NJ, b=BL)
            cofs = [
                coef_all[:, i * NJ * NL:(i + 1) * NJ * NL]
                .rearrange("p (j l) -> p j l", j=NJ)
                .broadcast_to([JS, NJ, NL, BL])
                for i in range(3)
            ]
            nc.vector.tensor_mul(resv, cofs[0], tA)
            nc.vector.tensor_mul(tmpv, cofs[1], tU1)
            nc.vector.tensor_add(res[:], res[:], tmp[:])
            nc.vector.tensor_mul(tmpv, cofs[2], tU0)
            nc.vector.tensor_add(res[:], res[:], tmp[:])

            nc.sync.dma_start(
                out=out.rearrange("(j p) c -> p j c", p=JS),
                in_=res[:, :].rearrange("p (j c) -> p j c", j=NJ))

    return nc


def _get_nc():
    global _cached
    if _cached is None:
        _cached = _build_nc()
        _cached.finalize()   # Bacc: runs reg alloc + codegen passes
    return _cached


def _host_coefs(alpha, fract, lambd, l):
    # All [12,...] fp32; compute in float64, cast at the end.
    a = alpha[:, 0].astype(np.float64)          # [12]
    f = fract[:, 0].astype(np.float64)          # [12]
    lam = lambd[:, 0, :, 0].astype(np.float64)  # [12, 200]
    ll = l[:, 0, :, 0].astype(np.float64)       # [12, 200]

    belta = np.zeros(NL)
    for la in range(NL):
        g_a1 = _gamma(a[la] + 1.0)
        belta[la] = sum(
            g_a1 / (_gamma(kk + 1.0) * _gamma(a[la] - kk + 1.0)) for kk in range(4)
        )
    cN = np.array([_gamma(a[la] + 1.0) / (6.0 * _gamma(a[la] - 2.0))
                   for la in range(NL)])

    # tile lambda/l from 200 -> 1200 (index n % 200)
    lam_t = np.tile(lam, (1, 6))                # [12, 1200]
    ll_t = np.tile(ll, (1, 6))                  # [12, 1200]

    inv_hf = (1.0 / H) ** f                     # 3**fract
    P = 2.0 * lam_t / belta[:, None] * inv_hf[:, None]
    Q = lam_t * ll_t / belta[:, None] / H
    R = Q * cN[:, None]
    return P.astype(np.float32), Q.astype(np.float32), R.astype(np.float32)


def kernel(A, WW, train_init, alpha, fract, lambd, l, A_y_list):
    from concourse.bass_utils import run_bass_kernel_spmd

    A = np.asarray(A, dtype=np.float32)
    WW = np.asarray(WW, dtype=np.float32)
    train_init = np.asarray(train_init, dtype=np.float32)

    P, Q, R = _host_coefs(
        np.asarray(alpha, np.float32), np.asarray(fract, np.float32),
        np.asarray(lambd, np.float32), np.asarray(l, np.float32))

    Wneg = -WW[:, :, 0]                         # [1200, 1200]

    xts, wcs, coefs = {}, {}, {}
    for beta in range(PB):
        bsl = slice(beta * BL, (beta + 1) * BL)
        xa = A[:, bsl, :, 0].transpose(2, 0, 1).reshape(N, CA)      # col=la*BL+b
        xu = train_init[bsl, :, :, 1].transpose(1, 2, 0).reshape(N, CU)  # col=t*BL+b
        XT = np.concatenate([xa, xu], axis=1)                       # [1200, 400]
        # partition-major: [KT, NK*C], col = k*C + c
        xts[beta] = np.ascontiguousarray(
            XT.reshape(NK, KT, C).transpose(1, 0, 2).reshape(KT, NK * C),
            dtype=np.float32)
    for g in range(PJ):
        gsl = slice(g * JL, (g + 1) * JL)
        # partition-major: [KT, NK*JL], col = k*JL + j
        wcs[g] = np.ascontiguousarray(
            Wneg[:, gsl].reshape(NK, KT, JL).transpose(1, 0, 2)
            .reshape(KT, NK * JL), dtype=np.float32)
        # coef [JS, 108]: col = kind*36 + jt*12 + la
        kinds = [M[:, gsl].reshape(NL, NJ, JS).transpose(2, 1, 0)
                 for M in (P, Q, R)]                                # [100, 3, 12]
        coefs[g] = np.ascontiguousarray(
            np.stack(kinds, axis=1).reshape(JS, 3 * NJ * NL), dtype=np.float32)

    in_maps = []
    for core in range(PB * PJ):
        beta, g = divmod(core, PJ)
        in_maps.append({"xt": xts[beta], "wc": wcs[g], "coef": coefs[g]})

    nc = _get_nc()
    res = run_bass_kernel_spmd(nc, in_maps, core_ids=list(range(PB * PJ)))
    kernel.last_results = res

    full = np.empty((B, NL, N), dtype=np.float32)
    for core in range(PB * PJ):
        beta, g = divmod(core, PJ)
        o = res.results[core]["out"]            # [300, 192], col = la*BL+b
        full[beta * BL:(beta + 1) * BL, :, g * JL:(g + 1) * JL] = (
            o.reshape(JL, NL, BL).transpose(2, 1, 0))
    return full.reshape(B, NL, N, 1)


# revision 13
# speedup vs baseline: 1.0046x; 1.0046x over previous
# Trainium2 Bass kernel for nn_FCM_series_1 (gnn_message_passing).
#
# Math (derived from the reference):
#   aggregate(X, WW)[l,b,j] = tanh(-sum_i X[l,b,i] * WW[i,j])
#   T_A  = aggregate(A, WW)                     (12 lags x B rows)
#   U[t] = aggregate(train_init[:,:,t,1], WW)   (13 unique rows per batch;
#          A_N_OLD[la] = U[la], A_0_NEW[la] = U[la+1])
#   out[b,la,j] = P[la,j]*T_A[la,b,j] + Q[la,j]*U[la+1,b,j] + R[la,j]*U[la,b,j]
# with host-computable coefficients
#   P[la,j] = 2 * lambd[la, j%200] / belta[la] * 3**fract[la]
#   Q[la,j] = 3 * lambd[la, j%200] * l[la, j%200] / belta[la]
#   R[la,j] = Q[la,j] * Gamma(a+1)/(6*Gamma(a-2))
#   belta[la] = sum_{k=0..3} Gamma(a+1)/(Gamma(k+1)*Gamma(a-k+1))
#
# Sharding over 8 cores: batch split x2 (16 each), output node dim j split x4
# (300 each). Per core one matmul chain: lhsT=W-chunk tiles, rhs=X^T tiles,
# PSUM-accumulated over 10 k-tiles of 120, in float32r (single-pass fp32 PE
# mode, 4x faster than fp32 LOW_HIGH). W is negated on the host so psum
# already holds -X@W; tanh on ScalarE; coefficient combine on VectorE with
# 0-stride broadcast APs; per-core [300,192] result re-assembled on the host.
#
# HBM layouts are host-repacked to partition-major so every DMA descriptor is
# one large contiguous run per partition; input DMAs are split between the two
# HWDGE queues (sync for W, scalar for X) to double aggregate DMA throughput.

import math

import numpy as np

LAG = 13
B = 32
N = 1200
H = 1.0 / 3.0

PB = 2          # batch shards
PJ = 4          # j shards
BL = B // PB    # 16 batches per core
JL = N // PJ    # 300 output nodes per core
NL = LAG - 1    # 12
CA = NL * BL    # 192 cols: T_A block, col = la*BL + b
CU = LAG * BL   # 208 cols: U block,  col = CA + t*BL + b
C = CA + CU     # 400 matmul moving cols
KT = 120        # contraction tile
NK = N // KT    # 10
JS = 100        # j subtile (psum partition dim)
NJ = JL // JS   # 3
NCH = 2         # input DMA chunks per tensor (5 k-tiles each)

_cached = None


def _gamma(x):
    return math.gamma(x)


def _build_nc():
    import concourse.bacc as bacc
    import concourse.mybir as mybir
    from concourse.tile import TileContext

    f32 = mybir.dt.float32
    f32r = mybir.dt.float32r
    nc = bacc.Bacc(None, target_bir_lowering=False)

    # partition-major repacked inputs (see kernel() for layouts)
    xt = nc.dram_tensor("xt", [KT, NK * C], f32r, kind="ExternalInput")
    wc = nc.dram_tensor("wc", [KT, NK * JL], f32r, kind="ExternalInput")
    coef = nc.dram_tensor("coef", [JS, 3 * NJ * NL], f32, kind="ExternalInput")
    out = nc.dram_tensor("out", [JL, CA], f32, kind="ExternalOutput")

    with TileContext(nc) as tc:
        with (
            tc.tile_pool(name="sb", bufs=1) as pool,
            tc.tile_pool(name="ps", bufs=1, space="PSUM") as pspool,
        ):
            # Balanced dual-queue input streaming: each HWDGE queue carries
            # half of W and half of X; k0.. arrives first on both queues.
            # groups: (tensor, k0, nk): W k0-2, W k3-4, X k5-7, X k8-9 on sync
            #         X k0-2, X k3-4, W k5-7, W k8-9 on scalar
            w_tiles = [None] * NK   # per-k SBUF views into group tiles
            x_tiles = [None] * NK
            groups = [
                (nc.sync,   [("w", 0, 3), ("w", 3, 2), ("x", 5, 3), ("x", 8, 2)]),
                (nc.scalar, [("x", 0, 3), ("x", 3, 2), ("w", 5, 3), ("w", 8, 2)]),
            ]
            gi = 0
            for eng, specs in groups:
                for kind, k0, nk in specs:
                    src, width, tl = (wc, JL, w_tiles) if kind == "w" \
                        else (xt, C, x_tiles)
                    g_tile = pool.tile([KT, nk * width], f32r,
                                       tag=f"g{gi}", name=f"g{gi}")
                    eng.dma_start(
                        out=g_tile[:],
                        in_=src[:, k0 * width:(k0 + nk) * width])
                    for kk in range(nk):
                        tl[k0 + kk] = g_tile[:, kk * width:(kk + 1) * width]
                    gi += 1
            coef_all = pool.tile([JS, 3 * NJ * NL], f32, tag="coef")
            nc.gpsimd.dma_start(out=coef_all[:], in_=coef[:, :])

            ps = [pspool.tile([JS, C], f32, tag=f"ps{jt}", name=f"ps{jt}")
                  for jt in range(NJ)]
            for k in range(NK):
                for jt in range(NJ):
                    nc.tensor.matmul(
                        ps[jt][:],
                        w_tiles[k][:, jt * JS:(jt + 1) * JS],
                        x_tiles[k],
                        start=(k == 0),
                        stop=(k == NK - 1),
                    )

            # Per-jt epilogue, pipelined: tanh on ACT, combine split across
            # DVE (jt0, jt2) and GpSimd (jt1), per-jt output DMA on
            # alternating queues.
            t_all = pool.tile([JS, NJ * C], f32, tag="t")
            res = pool.tile([JS, NJ * CA], f32, tag="res")
            tmp = pool.tile([JS, NJ * CA], f32, tag="tmp")
            out3 = out.rearrange("(j p) c -> p j c", p=JS)
            for jt in range(NJ):
                # W was negated on the host, so psum = -(X @ W) already.
                nc.scalar.activation(
                    out=t_all[:, jt * C:(jt + 1) * C], in_=ps[jt][:],
                    func=mybir.ActivationFunctionType.Tanh,
                )
                t0 = jt * C
                tA = t_all[:, t0:t0 + CA].rearrange("p (l b) -> p l b", b=BL)
                tU1 = t_all[:, t0 + CA + BL:t0 + CA + CU].rearrange(
                    "p (l b) -> p l b", b=BL)
                tU0 = t_all[:, t0 + CA:t0 + CA + CA].rearrange(
                    "p (l b) -> p l b", b=BL)
                r0 = jt * CA
                rs = res[:, r0:r0 + CA]
                ts = tmp[:, r0:r0 + CA]
                rv = rs.rearrange("p (l b) -> p l b", b=BL)
                tv = ts.rearrange("p (l b) -> p l b", b=BL)
                cof = [coef_all[:, i * NJ * NL + jt * NL:
                                i * NJ * NL + (jt + 1) * NL]
                       .broadcast_to([JS, NL, BL]) for i in range(3)]
                ve = nc.vector if jt != 1 else nc.gpsimd
                ve.tensor_mul(rv, cof[0], tA)
                ve.tensor_mul(tv, cof[1], tU1)
                ve.tensor_add(rs, rs, ts)
                ve.tensor_mul(tv, cof[2], tU0)
                ve.tensor_add(rs, rs, ts)
                oeng = nc.sync if jt != 1 else nc.scalar
                oeng.dma_start(out=out3[:, jt, :], in_=rs)

    return nc


def _get_nc():
    global _cached
    if _cached is None:
        _cached = _build_nc()
        _cached.finalize()   # Bacc: runs reg alloc + codegen passes
    return _cached


def _host_coefs(alpha, fract, lambd, l):
    # All [12,...] fp32; compute in float64, cast at the end.
    a = alpha[:, 0].astype(np.float64)          # [12]
    f = fract[:, 0].astype(np.float64)          # [12]
    lam = lambd[:, 0, :, 0].astype(np.float64)  # [12, 200]
    ll = l[:, 0, :, 0].astype(np.float64)       # [12, 200]

    belta = np.zeros(NL)
    for la in range(NL):
        g_a1 = _gamma(a[la] + 1.0)
        belta[la] = sum(
            g_a1 / (_gamma(kk + 1.0) * _gamma(a[la] - kk + 1.0)) for kk in range(4)
        )
    cN = np.array([_gamma(a[la] + 1.0) / (6.0 * _gamma(a[la] - 2.0))
                   for la in range(NL)])

    # tile lambda/l from 200 -> 1200 (index n % 200)
    lam_t = np.tile(lam, (1, 6))                # [12, 1200]
    ll_t = np.tile(ll, (1, 6))                  # [12, 1200]

    inv_hf = (1.0 / H) ** f                     # 3**fract
    P = 2.0 * lam_t / belta[:, None] * inv_hf[:, None]
    Q = lam_t * ll_t / belta[:, None] / H
    R = Q * cN[:, None]
    return P.astype(np.float32), Q.astype(np.float32), R.astype(np.float32)


def kernel(A, WW, train_init, alpha, fract, lambd, l, A_y_list):
    from concourse.bass_utils import run_bass_kernel_spmd

    A = np.asarray(A, dtype=np.float32)
    WW = np.asarray(WW, dtype=np.float32)
    train_init = np.asarray(train_init, dtype=np.float32)

    P, Q, R = _host_coefs(
        np.asarray(alpha, np.float32), np.asarray(fract, np.float32),
        np.asarray(lambd, np.float32), np.asarray(l, np.float32))

    Wneg = -WW[:, :, 0]                         # [1200, 1200]

    xts, wcs, coefs = {}, {}, {}
    for beta in range(PB):
        bsl = slice(beta * BL, (beta + 1) * BL)
        xa = A[:, bsl, :, 0].transpose(2, 0, 1).reshape(N, CA)      # col=la*BL+b
        xu = train_init[bsl, :, :, 1].transpose(1, 2, 0).reshape(N, CU)  # col=t*BL+b
        XT = np.concatenate([xa, xu], axis=1)                       # [1200, 400]
        # partition-major: [KT, NK*C], col = k*C + c
        xts[beta] = np.ascontiguousarray(
            XT.reshape(NK, KT, C).transpose(1, 0, 2).reshape(KT, NK * C),
            dtype=np.float32)
    for g in range(PJ):
        gsl = slice(g * JL, (g + 1) * JL)
        # partition-major: [KT, NK*JL], col = k*JL + j
        wcs[g] = np.ascontiguousarray(
            Wneg[:, gsl].reshape(NK, KT, JL).transpose(1, 0, 2)
            .reshape(KT, NK * JL), dtype=np.float32)
        # coef [JS, 108]: col = kind*36 + jt*12 + la
        kinds = [M[:, gsl].reshape(NL, NJ, JS).transpose(2, 1, 0)
                 for M in (P, Q, R)]                                # [100, 3, 12]
        coefs[g] = np.ascontiguousarray(
            np.stack(kinds, axis=1).reshape(JS, 3 * NJ * NL), dtype=np.float32)

    in_maps = []
    for core in range(PB * PJ):
        beta, g = divmod(core, PJ)
        in_maps.append({"xt": xts[beta], "wc": wcs[g], "coef": coefs[g]})

    nc = _get_nc()
    res = run_bass_kernel_spmd(nc, in_maps, core_ids=list(range(PB * PJ)))
    kernel.last_results = res

    full = np.empty((B, NL, N), dtype=np.float32)
    for core in range(PB * PJ):
        beta, g = divmod(core, PJ)
        o = res.results[core]["out"]            # [300, 192], col = la*BL+b
        full[beta * BL:(beta + 1) * BL, :, g * JL:(g + 1) * JL] = (
            o.reshape(JL, NL, BL).transpose(2, 1, 0))
    return full.reshape(B, NL, N, 1)
